# revision 10
# baseline (speedup 1.0000x reference)
"""HetConv (3x3 block-diagonal-by-residue + 1x1 elsewhere) on 8 trn2 cores.

Strategy: data-parallel over batch (4 images/core, weights replicated).
Per core: implicit-GEMM conv over a 66x66 zero-padded SBUF image with
channels permuted by residue mod 4 (done via strided DMA access patterns,
never materialized on host). Effective weight packs into 20 [128x128]
matmul slots per spatial tile:
  - slots 10c+ti, c in {0,1}, ti in 0..8: tap (ky,kx)=divmod(ti,3), block-diag
    Wk for groups (2c, 2c+1); center tap also carries same-chunk W1 in its
    off-diagonal blocks.
  - slot 10c+9: cross-chunk center-tap W1 (other chunk's ic -> chunk c's oc).

All tensors are bf16 (PSUM accumulation stays fp32): halves DMA traffic at
the same 1.0 cycle/row tensor-engine rate as fp32r. Startup latency is
hidden by loading image 0 in per-band row pieces (compute starts ~4us in)
while scratch warm-up matmuls keep the PE busy from t~0 so it is fully
p-state-ramped when the real matmuls dispatch.
"""
import sys

sys.path.insert(0, "/opt/trn_rl_repo")

import numpy as np
import ml_dtypes
import concourse.bacc as bacc
import concourse.mybir as mybir
from concourse import tile
from concourse.bass_utils import run_bass_kernel_spmd

N_CORES = 8
B, C, H, W = 32, 256, 64, 64
BP = B // N_CORES          # images per core
HP, WP = H + 2, W + 2      # padded image
NTILES = 8                 # output row-bands per image
RPT = H // NTILES          # rows per band
NFREE = RPT * W            # matmul moving free size (512)
NSLOTS = 20
NWARM = 36                 # warm-up matmuls (p-state ramp during startup DMA)
WARM_N = 128

_PROG = None


def _build(reps=1):
    nc = bacc.Bacc("TRN2", target_bir_lowering=False, debug=False,
                   num_devices=N_CORES)
    f32 = mybir.dt.float32
    bf16 = mybir.dt.bfloat16

    # x arrives host-padded to [BP, C, 66, 66] (zero border) so the whole
    # padded image DMAs as one contiguous run per partition.
    x = nc.dram_tensor("x", [BP, C, HP * WP], bf16, kind="ExternalInput").ap()
    w = nc.dram_tensor("w", [128, NSLOTS * 128], bf16, kind="ExternalInput").ap()
    out = nc.dram_tensor("out", [BP, C, H, W], bf16, kind="ExternalOutput").ap()

    # channel c = 4k + g  ->  [b, g, k, ...]
    x_r = x.rearrange("b (k four) s -> b four k s", four=4)
    out_r = out.rearrange("b (k four) h w -> b four k h w", four=4)

    with tile.TileContext(nc) as tc:
        with (
            tc.tile_pool(name="wpool", bufs=1) as wpool,
            tc.tile_pool(name="xpool", bufs=2) as xpool,
            tc.tile_pool(name="opool", bufs=3) as opool,
            tc.tile_pool(name="wmsb", bufs=1) as wmsb,
            tc.tile_pool(name="pspool", bufs=3, space="PSUM") as pspool,
            tc.tile_pool(name="wmps", bufs=1, space="PSUM") as wmps,
        ):
            # scratch for PE warm-up: tiny memset, then NWARM dummy matmuls
            # keep the tensor engine continuously busy from ~t0 while the
            # first image loads, so the p-state ramp (3us of busy streak to
            # reach 2.4GHz) completes before the first real matmul issues.
            wsrc = wmsb.tile([1, WARM_N], bf16)
            nc.gpsimd.memset(wsrc[:, :], 0.0)

            # weight slots land in two halves: slots 0-9 (all of band-0
            # oc-chunk-0's needs, incl. its cross-chunk W1 slot) first, so
            # the first real matmul only waits ~3us; slots 10-19 follow the
            # first row pieces.
            wt = wpool.tile([128, NSLOTS * 128], bf16)
            nc.sync.dma_start(out=wt[:, :10 * 128], in_=w[:, :10 * 128])

            # image 0 loads in per-band row pieces so band-0 compute can
            # start as soon as slots + ~10 rows are resident. Chunk 0 on the
            # ACT HWDGE path, chunk 1 on the Pool SWDGE path: descriptor
            # generation runs in parallel.
            xp0 = [xpool.tile([128, HP * WP], bf16, tag=f"xp{c}",
                              name=f"xp0_{c}") for c in (0, 1)]
            piece_rows = [(0, 10)] + [(8 * k + 2, 8 * k + 10) for k in range(1, NTILES)]
            for pi, (r0, r1) in enumerate(piece_rows):
                for cchunk, eng in ((0, nc.scalar), (1, nc.gpsimd)):
                    eng.dma_start(
                        out=xp0[cchunk][:, r0 * WP:r1 * WP],
                        in_=x_r[0, 2 * cchunk:2 * cchunk + 2, :, r0 * WP:r1 * WP],
                    )
                if pi == 0:
                    nc.sync.dma_start(out=wt[:, 10 * 128:],
                                      in_=w[:, 10 * 128:])

            wps = wmps.tile([1, WARM_N], f32)
            for _ in range(NWARM):
                nc.tensor.matmul(wps[:, :], wsrc[0:1, 0:1], wsrc[0:1, :],
                                 start=True, stop=True)

            def wslot(s):
                return wt[:, s * 128:(s + 1) * 128]

            for img in [i % BP for i in range(BP * reps)]:
                if img == 0:
                    xvs = [t[:, :].rearrange("p (h w) -> p h w", w=WP)
                           for t in xp0]
                else:
                    xvs = []
                    for cchunk in (0, 1):
                        xp = xpool.tile([128, HP * WP], bf16,
                                        tag=f"xp{cchunk}")
                        # partitions 0-63 <- residue 2c, 64-127 <- 2c+1; one
                        # fully-contiguous DMA per (img, chunk)
                        nc.gpsimd.dma_start(
                            out=xp[:, :],
                            in_=x_r[img, 2 * cchunk:2 * cchunk + 2],
                        )
                        xvs.append(xp[:, :].rearrange("p (h w) -> p h w", w=WP))

                for nt in range(NTILES):
                    def rhs(cchunk, ky, kx):
                        return xvs[cchunk][:, nt * RPT + ky: nt * RPT + ky + RPT,
                                           kx:kx + W]

                    for oc_chunk in (0, 1):
                        ps = pspool.tile([128, NFREE], f32, tag=f"ps{oc_chunk}")
                        for ti in range(9):
                            ky, kx = divmod(ti, 3)
                            nc.tensor.matmul(
                                ps[:, :], wslot(10 * oc_chunk + ti),
                                rhs(oc_chunk, ky, kx),
                                start=(ti == 0), stop=False,
                            )
                        # cross-chunk center-tap W1 (slot 10c+9: other
                        # chunk's ic -> this chunk's oc)
                        nc.tensor.matmul(
                            ps[:, :], wslot(10 * oc_chunk + 9),
                            rhs(1 - oc_chunk, 1, 1),
                            start=False, stop=True,
                        )
                        ot = opool.tile([128, NFREE], bf16, tag=f"ot{oc_chunk}")
                        nc.vector.tensor_copy(ot[:, :], ps[:, :])
                        # one DMA per (band, chunk): partitions (g, k) map to
                        # channels 4k+g for the chunk's two residue groups
                        eng = nc.sync if oc_chunk == 0 else nc.scalar
                        eng.dma_start(
                            out=out_r[img, 2 * oc_chunk:2 * oc_chunk + 2, :,
                                      nt * RPT:(nt + 1) * RPT, :],
                            in_=ot[:, :],
                        )

    nc.compile()
    return nc


def _get_prog():
    global _PROG
    if _PROG is None:
        _PROG = _build()
    return _PROG


def _prep_weights(Wk, W1):
    idx = [np.arange(g, 256, 4) for g in range(4)]
    wslabs = np.zeros((NSLOTS, 128, 128), np.float32)
    for c in (0, 1):
        gs = (2 * c, 2 * c + 1)
        for ti in range(9):
            ky, kx = divmod(ti, 3)
            s = 10 * c + ti
            for a in (0, 1):        # ic block position
                for b in (0, 1):    # oc block position
                    ga, gb = gs[a], gs[b]
                    if a == b:
                        blk = Wk[np.ix_(idx[gb], idx[ga])][:, :, ky, kx].T
                    elif ti == 4:
                        blk = W1[np.ix_(idx[gb], idx[ga])].T
                    else:
                        continue
                    wslabs[s, 64 * a:64 * a + 64, 64 * b:64 * b + 64] = blk
    for s, (ic_gs, oc_gs) in ((19, ((0, 1), (2, 3))), (9, ((2, 3), (0, 1)))):
        for a, ga in enumerate(ic_gs):
            for b, gb in enumerate(oc_gs):
                wslabs[s, 64 * a:64 * a + 64, 64 * b:64 * b + 64] = \
                    W1[np.ix_(idx[gb], idx[ga])].T
    # SBUF layout [K partition, slot*128 + m]
    return np.ascontiguousarray(
        wslabs.transpose(1, 0, 2).reshape(128, NSLOTS * 128)
    ).astype(ml_dtypes.bfloat16)


def _make_in_maps(x, Wk, W1):
    w_host = _prep_weights(np.asarray(Wk, np.float32), np.asarray(W1, np.float32))
    xs = np.asarray(x, np.float32)
    xpad = np.zeros((B, C, HP, WP), np.float32)
    xpad[:, :, 1:H + 1, 1:W + 1] = xs
    xpad = xpad.reshape(B, C, HP * WP).astype(ml_dtypes.bfloat16)
    return [
        {"x": np.ascontiguousarray(xpad[i * BP:(i + 1) * BP]), "w": w_host}
        for i in range(N_CORES)
    ]


def _run(x, Wk, W1, **spmd_kwargs):
    nc = _get_prog()
    in_maps = _make_in_maps(x, Wk, W1)
    res = run_bass_kernel_spmd(nc, in_maps, list(range(N_CORES)), **spmd_kwargs)
    outs = np.concatenate(
        [np.asarray(res.results[i]["out"]) for i in range(N_CORES)], axis=0)
    return outs.astype(np.float32), res


def kernel(x, Wk, W1):
    return _run(x, Wk, W1)[0]


# revision 11
# speedup vs baseline: 1.0241x; 1.0241x over previous
"""HetConv (3x3 block-diagonal-by-residue + 1x1 elsewhere) on 8 trn2 cores.

Strategy: data-parallel over batch (4 images/core, weights replicated).
Per core: implicit-GEMM conv over a 66x66 zero-padded SBUF image with
channels permuted by residue mod 4 (done via strided DMA access patterns,
never materialized on host). Effective weight packs into 20 [128x128]
matmul slots per spatial tile:
  - slots 10c+ti, c in {0,1}, ti in 0..8: tap (ky,kx)=divmod(ti,3), block-diag
    Wk for groups (2c, 2c+1); center tap also carries same-chunk W1 in its
    off-diagonal blocks.
  - slot 10c+9: cross-chunk center-tap W1 (other chunk's ic -> chunk c's oc).

All tensors are bf16 (PSUM accumulation stays fp32): halves DMA traffic at
the same 1.0 cycle/row tensor-engine rate as fp32r. Startup latency is
hidden by loading image 0 in per-band row pieces (compute starts ~4us in)
while scratch warm-up matmuls keep the PE busy from t~0 so it is fully
p-state-ramped when the real matmuls dispatch.
"""
import sys

sys.path.insert(0, "/opt/trn_rl_repo")

import numpy as np
import ml_dtypes
import concourse.bacc as bacc
import concourse.mybir as mybir
from concourse import tile
from concourse.bass_utils import run_bass_kernel_spmd

N_CORES = 8
B, C, H, W = 32, 256, 64, 64
BP = B // N_CORES          # images per core
HP, WP = H + 2, W + 2      # padded image
NTILES = 8                 # output row-bands per image
RPT = H // NTILES          # rows per band
NFREE = RPT * W            # matmul moving free size (512)
NSLOTS = 20
NWARM = 29                 # warm-up matmuls (p-state ramp during startup DMA)
WARM_N = 128

_PROG = None


def _build(reps=1):
    nc = bacc.Bacc("TRN2", target_bir_lowering=False, debug=False,
                   num_devices=N_CORES)
    f32 = mybir.dt.float32
    bf16 = mybir.dt.bfloat16

    # x arrives host-padded to [BP, C, 66, 66] (zero border) so the whole
    # padded image DMAs as one contiguous run per partition.
    x = nc.dram_tensor("x", [BP, C, HP * WP], bf16, kind="ExternalInput").ap()
    w = nc.dram_tensor("w", [128, NSLOTS * 128], bf16, kind="ExternalInput").ap()
    out = nc.dram_tensor("out", [BP, C, H, W], bf16, kind="ExternalOutput").ap()

    # channel c = 4k + g  ->  [b, g, k, ...]
    x_r = x.rearrange("b (k four) s -> b four k s", four=4)
    out_r = out.rearrange("b (k four) h w -> b four k h w", four=4)

    with tile.TileContext(nc) as tc:
        with (
            tc.tile_pool(name="wpool", bufs=1) as wpool,
            tc.tile_pool(name="xpool", bufs=2) as xpool,
            tc.tile_pool(name="opool", bufs=3) as opool,
            tc.tile_pool(name="wmsb", bufs=1) as wmsb,
            tc.tile_pool(name="pspool", bufs=3, space="PSUM") as pspool,
            tc.tile_pool(name="wmps", bufs=1, space="PSUM") as wmps,
        ):
            # scratch for PE warm-up: tiny memset, then NWARM dummy matmuls
            # keep the tensor engine continuously busy from ~t0 while the
            # first image loads, so the p-state ramp (3us of busy streak to
            # reach 2.4GHz) completes before the first real matmul issues.
            wsrc = wmsb.tile([1, WARM_N], bf16)
            nc.gpsimd.memset(wsrc[:, :], 0.0)

            # weight slots land in two halves: slots 0-9 (all of band-0
            # oc-chunk-0's needs, incl. its cross-chunk W1 slot) first, so
            # the first real matmul only waits ~3us; slots 10-19 follow the
            # first row pieces.
            wt = wpool.tile([128, NSLOTS * 128], bf16)
            nc.sync.dma_start(out=wt[:, :10 * 128], in_=w[:, :10 * 128])

            # image 0 loads in per-band row pieces so band-0 compute can
            # start as soon as slots + ~10 rows are resident. Chunk 0 on the
            # ACT HWDGE path, chunk 1 on the Pool SWDGE path: descriptor
            # generation runs in parallel.
            xp0 = [xpool.tile([128, HP * WP], bf16, tag=f"xp{c}",
                              name=f"xp0_{c}") for c in (0, 1)]
            piece_rows = [(0, 10)] + [(8 * k + 2, 8 * k + 10) for k in range(1, NTILES)]
            for pi, (r0, r1) in enumerate(piece_rows):
                for cchunk, eng in ((0, nc.scalar), (1, nc.gpsimd)):
                    eng.dma_start(
                        out=xp0[cchunk][:, r0 * WP:r1 * WP],
                        in_=x_r[0, 2 * cchunk:2 * cchunk + 2, :, r0 * WP:r1 * WP],
                    )
                if pi == 0:
                    nc.sync.dma_start(out=wt[:, 10 * 128:],
                                      in_=w[:, 10 * 128:])

            wps = wmps.tile([1, WARM_N], f32)
            for _ in range(NWARM):
                nc.tensor.matmul(wps[:, :], wsrc[0:1, 0:1], wsrc[0:1, :],
                                 start=True, stop=True)

            def wslot(s):
                return wt[:, s * 128:(s + 1) * 128]

            for img in [i % BP for i in range(BP * reps)]:
                if img == 0:
                    xvs = [t[:, :].rearrange("p (h w) -> p h w", w=WP)
                           for t in xp0]
                else:
                    xvs = []
                    for cchunk in (0, 1):
                        xp = xpool.tile([128, HP * WP], bf16,
                                        tag=f"xp{cchunk}")
                        # partitions 0-63 <- residue 2c, 64-127 <- 2c+1; one
                        # fully-contiguous DMA per (img, chunk)
                        nc.gpsimd.dma_start(
                            out=xp[:, :],
                            in_=x_r[img, 2 * cchunk:2 * cchunk + 2],
                        )
                        xvs.append(xp[:, :].rearrange("p (h w) -> p h w", w=WP))

                for nt in range(NTILES):
                    def rhs(cchunk, ky, kx):
                        return xvs[cchunk][:, nt * RPT + ky: nt * RPT + ky + RPT,
                                           kx:kx + W]

                    for oc_chunk in (0, 1):
                        ps = pspool.tile([128, NFREE], f32, tag=f"ps{oc_chunk}")
                        for ti in range(9):
                            ky, kx = divmod(ti, 3)
                            nc.tensor.matmul(
                                ps[:, :], wslot(10 * oc_chunk + ti),
                                rhs(oc_chunk, ky, kx),
                                start=(ti == 0), stop=False,
                            )
                        # cross-chunk center-tap W1 (slot 10c+9: other
                        # chunk's ic -> this chunk's oc)
                        nc.tensor.matmul(
                            ps[:, :], wslot(10 * oc_chunk + 9),
                            rhs(1 - oc_chunk, 1, 1),
                            start=False, stop=True,
                        )
                        ot = opool.tile([128, NFREE], bf16, tag=f"ot{oc_chunk}")
                        nc.vector.tensor_copy(ot[:, :], ps[:, :])
                        # one DMA per (band, chunk): partitions (g, k) map to
                        # channels 4k+g for the chunk's two residue groups
                        eng = nc.sync if oc_chunk == 0 else nc.scalar
                        eng.dma_start(
                            out=out_r[img, 2 * oc_chunk:2 * oc_chunk + 2, :,
                                      nt * RPT:(nt + 1) * RPT, :],
                            in_=ot[:, :],
                        )

    nc.compile()
    return nc


def _get_prog():
    global _PROG
    if _PROG is None:
        _PROG = _build()
    return _PROG


def _prep_weights(Wk, W1):
    idx = [np.arange(g, 256, 4) for g in range(4)]
    wslabs = np.zeros((NSLOTS, 128, 128), np.float32)
    for c in (0, 1):
        gs = (2 * c, 2 * c + 1)
        for ti in range(9):
            ky, kx = divmod(ti, 3)
            s = 10 * c + ti
            for a in (0, 1):        # ic block position
                for b in (0, 1):    # oc block position
                    ga, gb = gs[a], gs[b]
                    if a == b:
                        blk = Wk[np.ix_(idx[gb], idx[ga])][:, :, ky, kx].T
                    elif ti == 4:
                        blk = W1[np.ix_(idx[gb], idx[ga])].T
                    else:
                        continue
                    wslabs[s, 64 * a:64 * a + 64, 64 * b:64 * b + 64] = blk
    for s, (ic_gs, oc_gs) in ((19, ((0, 1), (2, 3))), (9, ((2, 3), (0, 1)))):
        for a, ga in enumerate(ic_gs):
            for b, gb in enumerate(oc_gs):
                wslabs[s, 64 * a:64 * a + 64, 64 * b:64 * b + 64] = \
                    W1[np.ix_(idx[gb], idx[ga])].T
    # SBUF layout [K partition, slot*128 + m]
    return np.ascontiguousarray(
        wslabs.transpose(1, 0, 2).reshape(128, NSLOTS * 128)
    ).astype(ml_dtypes.bfloat16)


def _make_in_maps(x, Wk, W1):
    w_host = _prep_weights(np.asarray(Wk, np.float32), np.asarray(W1, np.float32))
    xs = np.asarray(x, np.float32)
    xpad = np.zeros((B, C, HP, WP), np.float32)
    xpad[:, :, 1:H + 1, 1:W + 1] = xs
    xpad = xpad.reshape(B, C, HP * WP).astype(ml_dtypes.bfloat16)
    return [
        {"x": np.ascontiguousarray(xpad[i * BP:(i + 1) * BP]), "w": w_host}
        for i in range(N_CORES)
    ]


def _run(x, Wk, W1, **spmd_kwargs):
    nc = _get_prog()
    in_maps = _make_in_maps(x, Wk, W1)
    res = run_bass_kernel_spmd(nc, in_maps, list(range(N_CORES)), **spmd_kwargs)
    outs = np.concatenate(
        [np.asarray(res.results[i]["out"]) for i in range(N_CORES)], axis=0)
    return outs.astype(np.float32), res


def kernel(x, Wk, W1):
    return _run(x, Wk, W1)[0]
